# revision 4
# baseline (speedup 1.0000x reference)
"""Trainium2 Bass kernel v2 for nn_ChebychevInput.

out[b,o,s] = sum_{i,p<=256} (WM*coef[o,i,p]) * cos(p*arccos(x[b,i,s]))

Key layout: K-dim packed as 6 k-tiles x 128 rows = 768 = 3i x 256p
(p=1..256; the p=0 term is constant 1 and folds into a per-output bias
added during the PSUM drain). k-tile kt = 2*i + t holds rows
j -> p = 128*t + j + 1, all with the same input dim i.

Per-core pipeline (s-shard of 16384, both batches, 32 chunks of 1024):
  prologue: theta' = (pi/2 - arcsin(x)) * 2^16/(2pi) on flat [96,1024];
            th16 = round(theta') as int16 -> HBM scratch [6, 16384]
  bcast:    DMA-replicate th16 rows -> [128, 2048] tiles (granule = 2
            chunks), one per i, prefetched one granule ahead.  No GPSIMD
            (avoids the DVE/GpSimd shared SBUF port lock).
  DVE:      y32[:, kt] = int32(round(th16 * p + 2^14)); low halfword is
            exactly the phase mod 2^16 (HW converts round+wrap via i32).
  ACT:      T = sin(2pi/2^16 * y16) per 2-ktile granule -> fp16
  PE:       out[o,s] accumulated over 6 k-tiles, lhsT = W fp16
  DVE:      drain PSUM [128,1024] (2 banks) + bias -> SBUF, DMA out.
"""
import sys

sys.path.insert(0, "/opt/trn_rl_repo")

import numpy as np

BATCH = 2
INPUT_DIM = 3
N_SAMPLES = 131072
OUTPUT_DIM = 256
POLY_DEGREE = 256
N_CORES = 8
S_SHARD = N_SAMPLES // N_CORES  # 16384
SC = 1024                       # sample chunk
NSC = S_SHARD // SC             # 16 chunks per batch
NKT = 6
WEIGHT_MAGNITUDE = float(np.sqrt(6.0 / (INPUT_DIM * (POLY_DEGREE + 1))))
TWO16 = 65536.0

_compiled = {}


def _build():
    import concourse.tile as tile
    from concourse import bacc, mybir

    F32 = mybir.dt.float32
    F16 = mybir.dt.float16
    I32 = mybir.dt.int32
    I16 = mybir.dt.int16
    AF = mybir.ActivationFunctionType
    ALU = mybir.AluOpType

    nc = bacc.Bacc("TRN2", target_bir_lowering=False, debug=False)
    x_d = nc.dram_tensor("x", [BATCH, INPUT_DIM, S_SHARD], F32, kind="ExternalInput")
    w_d = nc.dram_tensor("w", [128, NKT * OUTPUT_DIM], F16, kind="ExternalInput")
    pc_d = nc.dram_tensor("pc", [128, 2], F32, kind="ExternalInput")
    bias_d = nc.dram_tensor("bias", [128, 2], F32, kind="ExternalInput")
    th16_h = nc.dram_tensor("th16h", [BATCH * INPUT_DIM, S_SHARD], I16)
    out_d = nc.dram_tensor("out", [BATCH, OUTPUT_DIM, S_SHARD], F32, kind="ExternalOutput")

    with tile.TileContext(nc) as tc:
        with (
            tc.tile_pool(name="const", bufs=1) as constp,
            tc.tile_pool(name="theta", bufs=1) as thp,
            tc.tile_pool(name="bcast", bufs=3) as bcp,
            tc.tile_pool(name="yint", bufs=3) as yp,
            tc.tile_pool(name="tmat", bufs=3) as tp,
            tc.tile_pool(name="outs", bufs=3) as op,
            tc.tile_pool(name="psum", bufs=2, space="PSUM") as pp,
        ):
            w_t = constp.tile([128, NKT * OUTPUT_DIM], F16)
            nc.sync.dma_start(w_t[:], w_d[:])
            pc_t = constp.tile([128, 2], F32)
            nc.sync.dma_start(pc_t[:], pc_d[:])
            bias_t = constp.tile([128, 2], F32)
            nc.sync.dma_start(bias_t[:], bias_d[:])

            # ---- theta stage: flat [96, 1024]; row = 48*b + 16*i + u
            # arccos(x) = 2*arctan(sqrt((1-x)/(1+x))), arctan arg in [0,1].
            # (1-x)/(1+x) = 2*sigmoid(-ln x) - 1  (avoids the slow DVE
            # reciprocal; x in (0,1) so ln is safe, x->0 degrades gracefully)
            xt = thp.tile([96, 1024], F32)
            nc.sync.dma_start(xt[:], x_d[:].rearrange("b i (u c) -> (b i u) c", c=1024))
            lnx = thp.tile([96, 1024], F32)
            nc.scalar.activation(lnx[:], xt[:], AF.Ln)
            sg = thp.tile([96, 1024], F32)
            nc.scalar.activation(sg[:], lnx[:], AF.Sigmoid, scale=-1.0)
            u2 = thp.tile([96, 1024], F32)
            nc.vector.tensor_scalar(u2[:], sg[:], 2.0, -1.0, ALU.mult, ALU.add)
            sr = thp.tile([96, 1024], F32)
            nc.scalar.activation(sr[:], u2[:], AF.Sqrt)
            asn = thp.tile([96, 1024], F32)
            nc.scalar.activation(asn[:], sr[:], AF.Arctan)
            # theta' = 2*arctan(.) * 2^16/(2pi) = arctan(.) * 2^17/(2pi)
            th16 = thp.tile([96, 1024], I16)
            nc.vector.tensor_scalar(
                th16[:], asn[:], float(2.0 * TWO16 / (2 * np.pi)), None, ALU.mult)
            nc.sync.dma_start(
                th16_h[:].rearrange("r (u c) -> (r u) c", c=1024), th16[:])

            # ---- main loop: 32 chunks; broadcast granule = 2 chunks
            def issue_bcasts(gi):
                # granule gi covers chunks 2*gi, 2*gi+1
                c0 = 2 * gi
                b = c0 // NSC
                s0 = (c0 % NSC) * SC
                tiles = {}
                for g in range(INPUT_DIM):
                    t = bcp.tile([128, 2 * SC], I16, tag=f"bc{g}")
                    nc.sync.dma_start(
                        t[:],
                        th16_h[3 * b + g: 3 * b + g + 1, s0: s0 + 2 * SC]
                        .unsqueeze(1).broadcast_to([1, 128, 2 * SC]))
                    tiles[g] = t
                return tiles

            bc_next = issue_bcasts(0)
            bc_cur = bc_next
            pending_drain = None

            def drain(item):
                b_, u_, ps_ = item
                for m in range(2):
                    ob = op.tile([128, 2 * 512], F32, tag=f"ob{m}", name=f"ob{m}")
                    nc.vector.tensor_scalar(
                        ob[:], ps_[m][:], bias_t[:, m:m + 1], None, ALU.add)
                    nc.sync.dma_start(
                        out_d[b_, m * 128:(m + 1) * 128, u_ * SC:(u_ + 1) * SC],
                        ob[:],
                    )

            for c in range(BATCH * NSC):
                b, u = c // NSC, c % NSC
                if c % 2 == 0:
                    bc_cur = bc_next
                    if c // 2 + 1 < BATCH * NSC // 2:
                        bc_next = issue_bcasts(c // 2 + 1)
                off = (c % 2) * SC
                ps = [pp.tile([128, 2 * 512], F32, tag=f"ps{m}", name=f"ps{m}") for m in range(2)]
                for g in range(INPUT_DIM):
                    bc = bc_cur[g]
                    y2 = yp.tile([128, 2 * SC], I32, tag=f"y{g}", name=f"y{g}")
                    nc.vector.tensor_scalar(
                        y2[:, 0:SC], bc[:, off:off + SC],
                        pc_t[:, 0:1], 16384.0, ALU.mult, ALU.add)
                    nc.vector.tensor_scalar(
                        y2[:, SC:2 * SC], bc[:, off:off + SC],
                        pc_t[:, 1:2], 16384.0, ALU.mult, ALU.add)
                    tm2 = tp.tile([128, 2 * SC], F16, tag=f"tm{g}", name=f"tm{g}")
                    yv = y2[:].bitcast(I16).rearrange(
                        "p (n two) -> p n two", two=2)[:, :, 0]
                    nc.scalar.activation(tm2[:], yv, AF.Sin,
                                         scale=float(2 * np.pi / TWO16))
                    if g == 1 and pending_drain is not None:
                        # previous chunk's PSUM drains, emitted after this
                        # chunk's first phase/Sin so they don't delay the
                        # next chunk's first matmul on the DVE queue
                        drain(pending_drain)
                        pending_drain = None
                    for t in range(2):
                        kt = 2 * g + t
                        for m in range(2):
                            for h in range(2):
                                nc.tensor.matmul(
                                    ps[m][:, h * 512:(h + 1) * 512],
                                    w_t[:, kt * OUTPUT_DIM + m * 128:
                                        kt * OUTPUT_DIM + m * 128 + 128],
                                    tm2[:, t * SC + h * 512: t * SC + h * 512 + 512],
                                    start=(kt == 0), stop=(kt == NKT - 1),
                                )
                pending_drain = (b, u, ps)
            drain(pending_drain)
    nc.compile()
    return nc


def _host_prep(coefficients):
    w = (np.asarray(coefficients, dtype=np.float64) * WEIGHT_MAGNITUDE).astype(np.float32)
    # wk[j, (2i+t)*256 + o] = w[o, i, 128t + j + 1]
    wk = np.zeros((128, NKT * OUTPUT_DIM), np.float32)
    j = np.arange(128)
    for i in range(INPUT_DIM):
        for t in range(2):
            kt = 2 * i + t
            p = 128 * t + j + 1
            wk[:, kt * OUTPUT_DIM:(kt + 1) * OUTPUT_DIM] = w[:, i, p].T
    pc = np.zeros((128, 2), np.float32)
    pc[:, 0] = j + 1
    pc[:, 1] = j + 129
    # bias[j, m] = sum_i w[m*128+j, i, 0]
    bias = np.zeros((128, 2), np.float32)
    for m in range(2):
        bias[:, m] = w[m * 128:(m + 1) * 128, :, 0].sum(axis=1)
    return wk.astype(np.float16), pc, bias


def _get_callable(n_execs=1):
    """Cached jitted shard_map callable running the bass program on 8 cores."""
    key = ("fn", n_execs)
    if key in _compiled:
        return _compiled[key]
    import jax
    from jax.sharding import Mesh, PartitionSpec
    from jax.experimental.shard_map import shard_map
    from concourse.bass2jax import (
        _bass_exec_p, install_neuronx_cc_hook, partition_id_tensor)

    if "nc" not in _compiled:
        _compiled["nc"] = _build()
    nc = _compiled["nc"]
    install_neuronx_cc_hook()

    in_names = ("x", "w", "pc", "bias", "out", "partition_id")
    out_names = ("out",)
    out_aval = jax.core.ShapedArray((BATCH, OUTPUT_DIM, S_SHARD), np.float32)

    def _body(xs, ws, pcs, bs, zs):
        outs = (zs,)
        for _ in range(n_execs):
            outs = _bass_exec_p.bind(
                xs, ws, pcs, bs, outs[0], partition_id_tensor(),
                out_avals=(out_aval,),
                in_names=in_names,
                out_names=out_names,
                lowering_input_output_aliases=(),
                sim_require_finite=True,
                sim_require_nnan=True,
                nc=nc,
            )
        return outs[0]

    devices = jax.devices()[:N_CORES]
    mesh = Mesh(np.asarray(devices), ("core",))
    fn = jax.jit(
        shard_map(
            _body, mesh=mesh,
            in_specs=(PartitionSpec("core"),) * 5,
            out_specs=PartitionSpec("core"),
            check_rep=False,
        ),
        donate_argnums=(4,),
        keep_unused=True,
    )
    _compiled[key] = fn
    return fn


def _make_zeros():
    import jax
    import jax.numpy as jnp
    from jax.sharding import Mesh, PartitionSpec, NamedSharding

    if "zmk" not in _compiled:
        devices = jax.devices()[:N_CORES]
        mesh = Mesh(np.asarray(devices), ("core",))
        sh = NamedSharding(mesh, PartitionSpec("core"))
        _compiled["zmk"] = jax.jit(
            lambda: jnp.zeros((N_CORES * BATCH, OUTPUT_DIM, S_SHARD), np.float32),
            out_shardings=sh)
    return _compiled["zmk"]()


def _core_sharding():
    import jax
    from jax.sharding import Mesh, PartitionSpec, NamedSharding

    if "sh" not in _compiled:
        devices = jax.devices()[:N_CORES]
        mesh = Mesh(np.asarray(devices), ("core",))
        _compiled["sh"] = NamedSharding(mesh, PartitionSpec("core"))
    return _compiled["sh"]


def _prep_globals(x, coefficients):
    wk, pc, bias = _host_prep(coefficients)
    xg = np.ascontiguousarray(
        np.asarray(x, dtype=np.float32).reshape(BATCH, INPUT_DIM, N_CORES, S_SHARD)
        .transpose(2, 0, 1, 3).reshape(N_CORES * BATCH, INPUT_DIM, S_SHARD))
    wg = np.tile(wk, (N_CORES, 1))
    pcg = np.tile(pc, (N_CORES, 1))
    biasg = np.tile(bias, (N_CORES, 1))
    return xg, wg, pcg, biasg


def kernel(x, coefficients):
    import jax

    fn = _get_callable(1)
    sh = _core_sharding()
    xg, wg, pcg, biasg = _prep_globals(x, coefficients)
    xg_d = jax.device_put(xg, sh)
    wg_d = jax.device_put(wg, sh)
    pcg_d = jax.device_put(pcg, sh)
    bg_d = jax.device_put(biasg, sh)
    out = fn(xg_d, wg_d, pcg_d, bg_d, _make_zeros())
    outh = np.asarray(out).reshape(N_CORES, BATCH, OUTPUT_DIM, S_SHARD)
    full = outh.transpose(1, 2, 0, 3).reshape(BATCH, OUTPUT_DIM, N_SAMPLES)
    return np.ascontiguousarray(full)
